# revision 10
# baseline (speedup 1.0000x reference)
"""Trainium2 Bass kernel for nn_CustomTransformerEncoderLayer_44676249813047.

Longformer-style encoder layer (B=2, S=8192, D=768, H=12, HD=64, W=256,
DFF=3072) with an outer pre/post residual + second FFN.

Sharding: sequence-parallel across 8 cores. Each core owns a contiguous
2048-token slice of one batch row and receives a W=256-token halo on each side
(zero-padded at sequence edges); the sliding-window attention is computed
locally with precomputed additive masks, so no collectives are needed.

Compute: matmuls in bf16 (fp32 PSUM accumulate), LayerNorm stats / softmax /
residuals in fp32. Scores are computed keys-major ([keys, queries]) so the
PV matmul consumes exp(scores) directly; the softmax denominator comes from a
ones-column appended to V, and normalization happens after a PE transpose of
the per-head output (where 1/sum is a per-partition scalar).
"""

import json

import ml_dtypes
import numpy as np

BF16 = ml_dtypes.bfloat16

B, S, D, H, HD, W, DFF = 2, 8192, 768, 12, 64, 256, 3072
NCORES = 8
T_OWN = (B * S) // NCORES  # 2048 tokens per core
MASK_NEG = -60.0
KC = D // 128  # 6 contraction chunks for D
KF = DFF // 128  # 24 chunks for DFF

_PATCHED = False


def _install_wait_split():
    """Public walrus codegen rejects instructions with >1 sync wait. Split the
    extra waits onto preceding NoOps on the same engine (the engine stalls
    in-order, so semantics are unchanged)."""
    global _PATCHED
    if _PATCHED:
        return
    import concourse.bass as bass

    orig = bass.Bass.to_json_bytes

    def patched(self):
        j = json.loads(orig(self))
        n = 0
        for fn in j.get("functions", []):
            for bb in fn.get("blocks", []):
                out = []
                for ins in bb.get("instructions", []):
                    si = ins.get("sync_info")
                    waits = (si or {}).get("on_wait") or []
                    if len(waits) > 1:
                        for w in waits[:-1]:
                            n += 1
                            out.append(
                                {
                                    "debug": ins.get("debug", 0),
                                    "engine": ins["engine"],
                                    "ins": [],
                                    "name": f"{ins['name']}-ws{n}",
                                    "opcode": "NoOp",
                                    "outs": [],
                                    "sync_info": {
                                        "on_update": [],
                                        "on_wait": [w],
                                    },
                                }
                            )
                        si["on_wait"] = waits[-1:]
                    out.append(ins)
                bb["instructions"] = out
        return json.dumps(j).encode()

    bass.Bass.to_json_bytes = patched
    _PATCHED = True


def build_nc(T, flags):
    """Build the single-core Bass program (SPMD: same program on all cores).

    T: number of own tokens for this build (T % 256 == 0). E = T + 2W.
    flags: dict of bools for optional bias/affine paths (host-inspected).
    """
    import concourse.bass as bass
    import concourse.mybir as mybir
    import concourse.tile as tile

    f32 = mybir.dt.float32
    bf16 = mybir.dt.bfloat16
    AF = mybir.ActivationFunctionType
    OP = mybir.AluOpType

    E = T + 2 * W
    NCH = T // 256  # query chunks
    NT = T // 128  # own 128-token tiles
    NTE = E // 128  # ext 128-token tiles
    TW = 512 if (T % 512 == 0 and E % 512 == 0) else 256  # wide token tile
    N512 = T // TW  # own wide token tiles
    N512E = E // TW
    NJ = TW // 128

    nc = bass.Bass()

    # ---- DRAM I/O ----
    src_ext = nc.dram_tensor("src_ext", [E, D], f32, kind="ExternalInput")
    pos_plus = nc.dram_tensor("pos_plus", [E, D], f32, kind="ExternalInput")
    w_qT = nc.dram_tensor("w_q", [D, D], bf16, kind="ExternalInput")  # wq/8
    w_k = nc.dram_tensor("w_k", [D, D], bf16, kind="ExternalInput")
    w_v = nc.dram_tensor("w_v", [D, D], bf16, kind="ExternalInput")
    w_o = nc.dram_tensor("w_o", [D, D], bf16, kind="ExternalInput")
    w_i = nc.dram_tensor("w_i", [D, DFF], bf16, kind="ExternalInput")
    w_o2 = nc.dram_tensor("w_o2", [DFF, D], bf16, kind="ExternalInput")
    w_1 = nc.dram_tensor("w_1", [D, DFF], bf16, kind="ExternalInput")  # n2-folded
    w_2 = nc.dram_tensor("w_2", [DFF, D], bf16, kind="ExternalInput")
    masks = nc.dram_tensor("masks", [NCH, 4, 128, 256], bf16, kind="ExternalInput")
    ident_b = nc.dram_tensor("ident_b", [128, 128], bf16, kind="ExternalInput")
    # bias columns (feature-major per-partition scalars), always passed (small)
    bq_c = nc.dram_tensor("bq_c", [128, KC], f32, kind="ExternalInput")  # bq/8
    bk_c = nc.dram_tensor("bk_c", [128, KC], f32, kind="ExternalInput")
    bi_c = nc.dram_tensor("bi_c", [128, KF], f32, kind="ExternalInput")
    b1_c = nc.dram_tensor("b1_c", [128, KF], f32, kind="ExternalInput")
    # bias rows for token-major outputs
    bv_r = nc.dram_tensor("bv_r", [1, D], bf16, kind="ExternalInput")
    bo_r = nc.dram_tensor("bo_r", [1, D], bf16, kind="ExternalInput")
    bo2_r = nc.dram_tensor("bo2_r", [1, D], bf16, kind="ExternalInput")
    b2_r = nc.dram_tensor("b2_r", [1, D], bf16, kind="ExternalInput")
    # LN affine broadcast tiles (used only when non-trivial)
    aff = {}
    for name in ("n1", "emb", "attn", "out"):
        if flags[f"aff_{name}"]:
            aff[name] = (
                nc.dram_tensor(f"{name}_s", [128, D], f32, kind="ExternalInput"),
                nc.dram_tensor(f"{name}_b", [128, D], f32, kind="ExternalInput"),
            )

    out_d = nc.dram_tensor("out", [T, D], f32, kind="ExternalOutput")
    src2_d = nc.dram_tensor("src2_scratch", [T, D], f32, kind="Internal")
    h_ext_d = nc.dram_tensor("h_ext_scratch", [E, D], bf16, kind="Internal")
    a_d = nc.dram_tensor("a_scratch", [T, D], bf16, kind="Internal")
    h2_d = nc.dram_tensor("h2_scratch", [T, D], bf16, kind="Internal")
    t2_d = nc.dram_tensor("t2_scratch", [T, D], bf16, kind="Internal")

    with tile.TileContext(nc) as tc:
        import contextlib

        ctx = contextlib.ExitStack()
        with ctx:
            const = ctx.enter_context(tc.tile_pool(name="const", bufs=1))

            # ---- constants ----
            idb = const.tile([128, 128], bf16, tag="idb")
            nc.sync.dma_start(idb[:], ident_b[:])
            eps5 = const.tile([128, 1], f32, tag="eps5")
            nc.vector.memset(eps5[:], 1e-5)
            eps12 = const.tile([128, 1], f32, tag="eps12")
            nc.vector.memset(eps12[:], 1e-12)
            bqc = const.tile([128, KC], f32, tag="bqc")
            nc.sync.dma_start(bqc[:], bq_c[:])
            bkc = const.tile([128, KC], f32, tag="bkc")
            nc.sync.dma_start(bkc[:], bk_c[:])
            bic = const.tile([128, KF], f32, tag="bic")
            nc.sync.dma_start(bic[:], bi_c[:])
            b1c = const.tile([128, KF], f32, tag="b1c")
            nc.sync.dma_start(b1c[:], b1_c[:])
            ones_r = const.tile([1, 128], bf16, tag="ones_r")
            nc.vector.memset(ones_r[:], 1.0)
            brow = {}
            for nm, dram in (
                ("bv", bv_r),
                ("bo", bo_r),
                ("bo2", bo2_r),
                ("b2", b2_r),
            ):
                if flags[nm]:
                    t = const.tile([1, D], bf16, tag=f"br_{nm}")
                    nc.sync.dma_start(t[:], dram[:])
                    brow[nm] = t
            aff_sb = {}
            for nm, (sd, bd) in aff.items():
                ts = const.tile([128, D], f32, tag=f"aff_s_{nm}")
                nc.sync.dma_start(ts[:], sd[:])
                tb = const.tile([128, D], f32, tag=f"aff_b_{nm}")
                nc.sync.dma_start(tb[:], bd[:])
                aff_sb[nm] = (ts, tb)

            def ln(pool, x_ap, eps_tile, out_ap, aff_name):
                """LayerNorm over free dim D; writes normalized (+affine) to
                out_ap. x_ap must be SBUF f32 [128, D]."""
                stats = pool.tile([128, 3, 6], f32, tag="ln_stats")
                for g in range(3):
                    nc.vector.bn_stats(
                        stats[:, g, :], x_ap[:, 256 * g : 256 * (g + 1)]
                    )
                mv = pool.tile([128, 2], f32, tag="ln_mv")
                nc.vector.bn_aggr(mv[:], stats[:])
                nc.scalar.activation(
                    mv[:, 1:2], mv[:, 1:2], AF.Sqrt, bias=eps_tile[:], scale=1.0
                )
                nc.vector.reciprocal(mv[:, 1:2], mv[:, 1:2])
                nc.vector.tensor_scalar(
                    out_ap,
                    x_ap,
                    mv[:, 0:1],
                    mv[:, 1:2],
                    OP.subtract,
                    OP.mult,
                )
                if aff_name is not None and aff_name in aff_sb:
                    ts, tb = aff_sb[aff_name]
                    nc.vector.tensor_mul(out_ap, out_ap, ts[:])
                    nc.vector.tensor_add(out_ap, out_ap, tb[:])

            def bias_row_mm(ps_ap, nm, n0, n1):
                """Accumulate a [1,D] bias row into PSUM via K=1 matmul."""
                if flags[nm]:
                    nc.tensor.matmul(
                        ps_ap[:, n0:n1],
                        lhsT=ones_r[:, : ps_ap.shape[0]],
                        rhs=brow[nm][:, n0:n1],
                        start=False,
                        stop=True,
                    )

            # attention operand pool (qT/kT/v), closes after P3 —
            # opened before p_hT so releases stay LIFO
            att_ctx = contextlib.ExitStack()
            p_att = att_ctx.enter_context(tc.tile_pool(name="p_att", bufs=1))
            # h^T ext pool (feature-major bf16), closes after P2
            hT_ctx = contextlib.ExitStack()
            p_hT = hT_ctx.enter_context(tc.tile_pool(name="p_hT", bufs=1))
            hT = [p_hT.tile([128, E], bf16, tag=f"hT_{j}", name=f"hT_{j}") for j in range(KC)]

            # ================= Phase 1: embeddings + LN1 + LN_emb ==========
            with tc.tile_pool(name="ph1", bufs=4) as ep:
                for i in range(NTE):
                    x = ep.tile([128, D], f32, tag="x")
                    nc.sync.dma_start(x[:], src_ext[128 * i : 128 * (i + 1), :])
                    p = ep.tile([128, D], f32, tag="p")
                    nc.sync.dma_start(p[:], pos_plus[128 * i : 128 * (i + 1), :])
                    hid = ep.tile([128, D], f32, tag="hid")
                    ln(ep, x[:], eps5, hid[:], "n1")
                    z = ep.tile([128, D], f32, tag="z")
                    nc.gpsimd.tensor_add(z[:], hid[:], p[:])
                    hb = ep.tile([128, D], bf16, tag="hb")
                    ln(ep, z[:], eps12, hb[:], "emb")
                    nc.sync.dma_start(h_ext_d[128 * i : 128 * (i + 1), :], hb[:])
                # transpose h via DMA xbar into feature-major
                for j in range(KC):
                    nc.sync.dma_start_transpose(
                        hT[j][:], h_ext_d[:, 128 * j : 128 * (j + 1)]
                    )

            qT = [p_att.tile([128, T], bf16, tag=f"qT_{j}", name=f"qT_{j}") for j in range(KC)]
            kT = [p_att.tile([128, E], bf16, tag=f"kT_{j}", name=f"kT_{j}") for j in range(KC)]
            v_sb = [
                p_att.tile([128, H, HD + 1], bf16, tag=f"v_{t}", name=f"v_{t}") for t in range(NTE)
            ]

            # ================= Phase 2: QKV projections ====================
            with tc.tile_pool(name="ph2w", bufs=1) as wp, tc.tile_pool(
                name="ph2ps", bufs=2, space="PSUM"
            ) as psq, tc.tile_pool(name="ph2psv", bufs=2, space="PSUM") as psv_p:
                wq_sb = [wp.tile([128, D], bf16, tag=f"wq_{k}", name=f"wq_{k}") for k in range(KC)]
                wk_sb = [wp.tile([128, D], bf16, tag=f"wk_{k}", name=f"wk_{k}") for k in range(KC)]
                wv_sb = [wp.tile([128, D], bf16, tag=f"wv_{k}", name=f"wv_{k}") for k in range(KC)]
                for k in range(KC):
                    nc.sync.dma_start(wq_sb[k][:], w_qT[128 * k : 128 * (k + 1), :])
                    nc.sync.dma_start(wk_sb[k][:], w_k[128 * k : 128 * (k + 1), :])
                    nc.sync.dma_start(wv_sb[k][:], w_v[128 * k : 128 * (k + 1), :])

                # q^T (own tokens), k^T (ext tokens): form (b)
                for m in range(KC):
                    for t in range(N512):
                        ps = psq.tile([128, TW], f32, tag="psq")
                        for k in range(KC):
                            nc.tensor.matmul(
                                ps[:],
                                lhsT=wq_sb[k][:, 128 * m : 128 * (m + 1)],
                                rhs=hT[k][:, W + TW * t : W + TW * (t + 1)],
                                start=(k == 0),
                                stop=(k == KC - 1),
                            )
                        if flags["bq"]:
                            nc.vector.tensor_scalar_add(
                                qT[m][:, TW * t : TW * (t + 1)],
                                ps[:],
                                bqc[:, m : m + 1],
                            )
                        else:
                            nc.vector.tensor_copy(
                                qT[m][:, TW * t : TW * (t + 1)], ps[:]
                            )
                    for t in range(N512E):
                        ps = psq.tile([128, TW], f32, tag="psq")
                        for k in range(KC):
                            nc.tensor.matmul(
                                ps[:],
                                lhsT=wk_sb[k][:, 128 * m : 128 * (m + 1)],
                                rhs=hT[k][:, TW * t : TW * (t + 1)],
                                start=(k == 0),
                                stop=(k == KC - 1),
                            )
                        if flags["bk"]:
                            nc.vector.tensor_scalar_add(
                                kT[m][:, TW * t : TW * (t + 1)],
                                ps[:],
                                bkc[:, m : m + 1],
                            )
                        else:
                            nc.vector.tensor_copy(
                                kT[m][:, TW * t : TW * (t + 1)], ps[:]
                            )

                # v token-major: form (a)
                for t in range(NTE):
                    ps = psv_p.tile([128, D], f32, tag="psv")
                    for k in range(KC):
                        nc.tensor.matmul(
                            ps[:, 0:512],
                            lhsT=hT[k][:, 128 * t : 128 * (t + 1)],
                            rhs=wv_sb[k][:, 0:512],
                            start=(k == 0),
                            stop=(k == KC - 1 and not flags["bv"]),
                        )
                        nc.tensor.matmul(
                            ps[:, 512:D],
                            lhsT=hT[k][:, 128 * t : 128 * (t + 1)],
                            rhs=wv_sb[k][:, 512:D],
                            start=(k == 0),
                            stop=(k == KC - 1 and not flags["bv"]),
                        )
                    bias_row_mm(ps, "bv", 0, 512)
                    bias_row_mm(ps, "bv", 512, D)
                    nc.vector.tensor_copy(
                        v_sb[t][:, :, 0:HD],
                        ps[:].rearrange("p (h d) -> p h d", h=H),
                    )
                    nc.gpsimd.memset(v_sb[t][:, :, HD : HD + 1], 1.0)

            hT_ctx.close()  # free h^T

            # ================= Phase 3: sliding-window attention ===========
            with tc.tile_pool(name="ph3m", bufs=2) as mp, tc.tile_pool(
                name="ph3p", bufs=4
            ) as pp, tc.tile_pool(name="ph3s", bufs=2, space="PSUM") as pss, tc.tile_pool(
                name="ph3a", bufs=1, space="PSUM"
            ) as psa, tc.tile_pool(name="ph3t", bufs=1, space="PSUM") as pstp:
                for cc in range(NCH):
                    msk = mp.tile([128, 4, 256], bf16, tag="msk")
                    nc.sync.dma_start(
                        msk[:], masks[cc].rearrange("b p q -> p b q")
                    )
                    a_t = [
                        pp.tile([128, D], bf16, tag=f"a_u{u}", name=f"a_u{u}")
                        for u in range(2)
                    ]
                    for hp in range(H // 2):
                        ps_pair = [
                            pss.tile([128, 6, 256], f32, tag="ps_s", name=f"ps_s{_h}")
                            for _h in range(2)
                        ]
                        # interleave the two heads' QK matmuls: adjacent
                        # matmuls target row groups 0-1 / 2-3 and overlap
                        for b in range(6):
                            key0 = cc * 256 + 128 * b
                            for hh in range(2):
                                r0 = 64 * hh
                                nc.tensor.matmul(
                                    ps_pair[hh][:, b, :],
                                    lhsT=kT[hp][r0 : r0 + 64, key0 : key0 + 128],
                                    rhs=qT[hp][
                                        r0 : r0 + 64, cc * 256 : cc * 256 + 256
                                    ],
                                    start=True,
                                    stop=True,
                                )
                        for hh in range(2):
                            h = 2 * hp + hh
                            ps_s = ps_pair[hh]
                            probs = pp.tile([128, 6, 256], bf16, tag="probs")
                            nc.scalar.activation(probs[:], ps_s[:], AF.Exp)
                            # multiplicative 0/1 band+validity masks
                            nc.vector.tensor_mul(
                                probs[:, 0:2, :], probs[:, 0:2, :], msk[:, 0:2, :]
                            )
                            nc.vector.tensor_mul(
                                probs[:, 4:6, :], probs[:, 4:6, :], msk[:, 2:4, :]
                            )
                            ps_a = psa.tile([HD + 1, 256], f32, tag="ps_a")
                            for b in range(6):
                                nc.tensor.matmul(
                                    ps_a[:],
                                    lhsT=v_sb[2 * cc + b][:, h, :],
                                    rhs=probs[:, b, :],
                                    start=(b == 0),
                                    stop=(b == 5),
                                )
                            aT = pp.tile([HD + 1, 256], bf16, tag="aT")
                            nc.vector.tensor_copy(aT[:], ps_a[:])
                            for u in range(2):
                                ps_t = pstp.tile([128, HD + 1], bf16, tag="ps_t")
                                nc.tensor.transpose(
                                    ps_t[:],
                                    aT[:, 128 * u : 128 * (u + 1)],
                                    idb[: HD + 1, : HD + 1],
                                )
                                rt = pp.tile([128, 1], f32, tag="rt")
                                nc.vector.reciprocal(rt[:], ps_t[:, HD : HD + 1])
                                nc.vector.tensor_scalar_mul(
                                    a_t[u][:, HD * h : HD * (h + 1)],
                                    ps_t[:, 0:HD],
                                    rt[:],
                                )
                    for u in range(2):
                        nc.sync.dma_start(
                            a_d[128 * (2 * cc + u) : 128 * (2 * cc + u + 1), :],
                            a_t[u][:],
                        )

            att_ctx.close()  # free qT/kT/v

            # h2^T persistent until end of phase 5
            h2T_ctx = contextlib.ExitStack()
            p_h2 = h2T_ctx.enter_context(tc.tile_pool(name="p_h2", bufs=1))
            h2T = [p_h2.tile([128, T], bf16, tag=f"h2T_{j}", name=f"h2T_{j}") for j in range(KC)]

            # ================= Phase 4: wo + residual + LN_attn ============
            with tc.tile_pool(name="ph4w", bufs=1) as wp4, tc.tile_pool(
                name="ph4", bufs=4
            ) as sp4, tc.tile_pool(name="ph4po", bufs=2, space="PSUM") as ps4o:
                wo_sb = [wp4.tile([128, D], bf16, tag=f"wo_{k}", name=f"wo_{k}") for k in range(KC)]
                for k in range(KC):
                    nc.sync.dma_start(wo_sb[k][:], w_o[128 * k : 128 * (k + 1), :])
                aT_sb = [
                    wp4.tile([128, T], bf16, tag=f"aTf_{j}", name=f"aTf_{j}")
                    for j in range(KC)
                ]
                for j in range(KC):
                    nc.sync.dma_start_transpose(
                        aT_sb[j][:], a_d[:, 128 * j : 128 * (j + 1)]
                    )
                for t in range(NT):
                    ps_o = ps4o.tile([128, D], f32, tag="ps_o")
                    for k in range(KC):
                        nc.tensor.matmul(
                            ps_o[:, 0:512],
                            lhsT=aT_sb[k][:, 128 * t : 128 * (t + 1)],
                            rhs=wo_sb[k][:, 0:512],
                            start=(k == 0),
                            stop=(k == KC - 1 and not flags["bo"]),
                        )
                        nc.tensor.matmul(
                            ps_o[:, 512:D],
                            lhsT=aT_sb[k][:, 128 * t : 128 * (t + 1)],
                            rhs=wo_sb[k][:, 512:D],
                            start=(k == 0),
                            stop=(k == KC - 1 and not flags["bo"]),
                        )
                    bias_row_mm(ps_o, "bo", 0, 512)
                    bias_row_mm(ps_o, "bo", 512, D)
                    h_t = sp4.tile([128, D], bf16, tag="h_t")
                    nc.sync.dma_start(
                        h_t[:], h_ext_d[W + 128 * t : W + 128 * (t + 1), :]
                    )
                    z2 = sp4.tile([128, D], f32, tag="z2")
                    nc.vector.tensor_add(z2[:], h_t[:], ps_o[:])
                    h2_t = sp4.tile([128, D], bf16, tag="h2_t")
                    ln(sp4, z2[:], eps12, h2_t[:], "attn")
                    nc.sync.dma_start(h2_d[128 * t : 128 * (t + 1), :], h2_t[:])
                for j in range(KC):
                    nc.sync.dma_start_transpose(
                        h2T[j][:], h2_d[:, 128 * j : 128 * (j + 1)]
                    )

            # ================= Phase 5: FFN (wi/gelu/wo2) + LN_out + src2 ==
            with tc.tile_pool(name="ph5w", bufs=1) as wp5, tc.tile_pool(
                name="ph5i", bufs=2
            ) as ip5, tc.tile_pool(name="ph5", bufs=3) as sp5, tc.tile_pool(
                name="ph5ps", bufs=3, space="PSUM"
            ) as ps5, tc.tile_pool(name="ph5pf", bufs=2, space="PSUM") as ps5f:
                wi_sb = [wp5.tile([128, DFF], bf16, tag=f"wi_{k}", name=f"wi_{k}") for k in range(KC)]
                wo2_sb = [
                    wp5.tile([128, D], bf16, tag=f"wo2_{k}", name=f"wo2_{k}") for k in range(KF)
                ]
                for k in range(KC):
                    nc.sync.dma_start(wi_sb[k][:], w_i[128 * k : 128 * (k + 1), :])
                for k in range(KF):
                    nc.sync.dma_start(wo2_sb[k][:], w_o2[128 * k : 128 * (k + 1), :])
                for t in range(N512):
                    interT = ip5.tile([128, KF, TW], bf16, tag="interT")
                    for m in range(KF):
                        ps = ps5.tile([128, TW], f32, tag="ps_i")
                        for k in range(KC):
                            nc.tensor.matmul(
                                ps[:],
                                lhsT=wi_sb[k][:, 128 * m : 128 * (m + 1)],
                                rhs=h2T[k][:, TW * t : TW * (t + 1)],
                                start=(k == 0),
                                stop=(k == KC - 1),
                            )
                        nc.scalar.activation(
                            interT[:, m, :],
                            ps[:],
                            AF.Gelu,
                            bias=bic[:, m : m + 1] if flags["bi"] else 0.0,
                            scale=1.0,
                        )
                    for j in range(NJ):
                        tt = NJ * t + j
                        ps_f = ps5f.tile([128, D], f32, tag="ps_f")
                        for k in range(KF):
                            nc.tensor.matmul(
                                ps_f[:, 0:512],
                                lhsT=interT[:, k, 128 * j : 128 * (j + 1)],
                                rhs=wo2_sb[k][:, 0:512],
                                start=(k == 0),
                                stop=(k == KF - 1 and not flags["bo2"]),
                            )
                            nc.tensor.matmul(
                                ps_f[:, 512:D],
                                lhsT=interT[:, k, 128 * j : 128 * (j + 1)],
                                rhs=wo2_sb[k][:, 512:D],
                                start=(k == 0),
                                stop=(k == KF - 1 and not flags["bo2"]),
                            )
                        bias_row_mm(ps_f, "bo2", 0, 512)
                        bias_row_mm(ps_f, "bo2", 512, D)
                        h2_t = sp5.tile([128, D], bf16, tag="h2_t5")
                        nc.sync.dma_start(
                            h2_t[:], h2_d[128 * tt : 128 * (tt + 1), :]
                        )
                        z3 = sp5.tile([128, D], f32, tag="z3")
                        nc.vector.tensor_add(z3[:], h2_t[:], ps_f[:])
                        outl = sp5.tile([128, D], f32, tag="outl")
                        ln(sp5, z3[:], eps12, outl[:], "out")
                        srct = sp5.tile([128, D], f32, tag="srct")
                        nc.sync.dma_start(
                            srct[:],
                            src_ext[W + 128 * tt : W + 128 * (tt + 1), :],
                        )
                        src2 = sp5.tile([128, D], f32, tag="src2")
                        nc.vector.tensor_add(src2[:], srct[:], outl[:])
                        nc.sync.dma_start(
                            src2_d[128 * tt : 128 * (tt + 1), :], src2[:]
                        )

            h2T_ctx.close()  # free h2T

            # ================= Phase 6: LN_n2 + transpose t2 ===============
            t2T_ctx = contextlib.ExitStack()
            p_t2T = t2T_ctx.enter_context(tc.tile_pool(name="p_t2T", bufs=1))
            t2T = [p_t2T.tile([128, T], bf16, tag=f"t2T_{j}", name=f"t2T_{j}") for j in range(KC)]
            with tc.tile_pool(name="ph6", bufs=3) as sp6:
                for t in range(NT):
                    s2 = sp6.tile([128, D], f32, tag="s2")
                    nc.sync.dma_start(
                        s2[:], src2_d[128 * t : 128 * (t + 1), :]
                    )
                    t2 = sp6.tile([128, D], bf16, tag="t2")
                    # n2 affine folded into w_1 on host -> no affine here
                    ln(sp6, s2[:], eps5, t2[:], None)
                    nc.sync.dma_start(t2_d[128 * t : 128 * (t + 1), :], t2[:])
                for j in range(KC):
                    nc.sync.dma_start_transpose(
                        t2T[j][:], t2_d[:, 128 * j : 128 * (j + 1)]
                    )

            # ================= Phase 7: FFN2 (w1/relu/w2) + final ==========
            with tc.tile_pool(name="ph7w", bufs=1) as wp7, tc.tile_pool(
                name="ph7i", bufs=2
            ) as ip7, tc.tile_pool(name="ph7", bufs=3) as sp7, tc.tile_pool(
                name="ph7ps", bufs=3, space="PSUM"
            ) as ps7, tc.tile_pool(name="ph7pf", bufs=2, space="PSUM") as ps7f:
                w1_sb = [wp7.tile([128, DFF], bf16, tag=f"w1_{k}", name=f"w1_{k}") for k in range(KC)]
                w2_sb = [wp7.tile([128, D], bf16, tag=f"w2_{k}", name=f"w2_{k}") for k in range(KF)]
                for k in range(KC):
                    nc.sync.dma_start(w1_sb[k][:], w_1[128 * k : 128 * (k + 1), :])
                for k in range(KF):
                    nc.sync.dma_start(w2_sb[k][:], w_2[128 * k : 128 * (k + 1), :])
                for t in range(N512):
                    reluT = ip7.tile([128, KF, TW], bf16, tag="reluT")
                    for m in range(KF):
                        ps = ps7.tile([128, TW], f32, tag="ps_r")
                        for k in range(KC):
                            nc.tensor.matmul(
                                ps[:],
                                lhsT=w1_sb[k][:, 128 * m : 128 * (m + 1)],
                                rhs=t2T[k][:, TW * t : TW * (t + 1)],
                                start=(k == 0),
                                stop=(k == KC - 1),
                            )
                        nc.scalar.activation(
                            reluT[:, m, :],
                            ps[:],
                            AF.Relu,
                            bias=b1c[:, m : m + 1] if flags["b1"] else 0.0,
                            scale=1.0,
                        )
                    for j in range(NJ):
                        tt = NJ * t + j
                        ps_f = ps7f.tile([128, D], f32, tag="ps_f7")
                        for k in range(KF):
                            nc.tensor.matmul(
                                ps_f[:, 0:512],
                                lhsT=reluT[:, k, 128 * j : 128 * (j + 1)],
                                rhs=w2_sb[k][:, 0:512],
                                start=(k == 0),
                                stop=(k == KF - 1 and not flags["b2"]),
                            )
                            nc.tensor.matmul(
                                ps_f[:, 512:D],
                                lhsT=reluT[:, k, 128 * j : 128 * (j + 1)],
                                rhs=w2_sb[k][:, 512:D],
                                start=(k == 0),
                                stop=(k == KF - 1 and not flags["b2"]),
                            )
                        bias_row_mm(ps_f, "b2", 0, 512)
                        bias_row_mm(ps_f, "b2", 512, D)
                        s2 = sp7.tile([128, D], f32, tag="s2b")
                        nc.sync.dma_start(
                            s2[:], src2_d[128 * tt : 128 * (tt + 1), :]
                        )
                        fin = sp7.tile([128, D], f32, tag="fin")
                        nc.vector.tensor_add(fin[:], s2[:], ps_f[:])
                        nc.sync.dma_start(
                            out_d[128 * tt : 128 * (tt + 1), :], fin[:]
                        )

            t2T_ctx.close()

    return nc


def _build_masks(T, core_start, valid_lo, valid_hi):
    """Additive masks per (chunk, masked-block) in keys-major layout
    [NCH, 4, 128 keys, 256 queries]. core_start: global position of own token
    0; valid range of global key positions [valid_lo, valid_hi)."""
    NCH = T // 256
    MASKED = (0, 1, 4, 5)
    m = np.zeros((NCH, 4, 128, 256), np.float32)
    i = np.arange(256)[None, :]  # query within chunk
    for cc in range(NCH):
        for bi_, b in enumerate(MASKED):
            u = 128 * b + np.arange(128)[:, None]  # key within chunk window
            band = (u >= i) & (u <= i + 2 * W)
            kpos = core_start - W + cc * 256 + u
            ok = band & (kpos >= valid_lo) & (kpos < valid_hi)
            m[cc, bi_] = np.where(ok, 1.0, 0.0)
    return m.astype(BF16)


def _host_prep(inputs, T, core_id):
    """Build the per-core input map."""
    E = T + 2 * W
    b = core_id // (NCORES // B)
    pos0 = (core_id % (NCORES // B)) * T

    def padded_slice(arr2d):
        out = np.zeros((E, D), np.float32)
        lo, hi = pos0 - W, pos0 + T + W
        clo, chi = max(lo, 0), min(hi, S)
        out[clo - lo : chi - lo] = arr2d[clo:chi]
        return out

    src_ext = padded_slice(np.asarray(inputs["src"][b], np.float32))
    pos_all = np.asarray(inputs["pos_emb"], np.float32) + np.asarray(
        inputs["tt_emb"], np.float32
    )[None, :]
    pos_plus = padded_slice(pos_all)

    f32 = np.float32
    n2_s = np.asarray(inputs["n2_s"], f32)
    n2_b = np.asarray(inputs["n2_b"], f32)
    w1 = np.asarray(inputs["w1"], f32)
    w1f = (n2_s[:, None] * w1).astype(BF16)
    b1f = (n2_b @ w1 + np.asarray(inputs["b1"], f32)).astype(f32)

    def col(v, kn):
        return np.ascontiguousarray(
            np.asarray(v, f32).reshape(kn, 128).T
        )  # [128, kn]

    im = {
        "src_ext": src_ext,
        "pos_plus": pos_plus,
        "w_q": (np.asarray(inputs["wq"], f32) / np.sqrt(HD)).astype(BF16),
        "w_k": np.asarray(inputs["wk"], f32).astype(BF16),
        "w_v": np.asarray(inputs["wv"], f32).astype(BF16),
        "w_o": np.asarray(inputs["wo"], f32).astype(BF16),
        "w_i": np.asarray(inputs["wi"], f32).astype(BF16),
        "w_o2": np.asarray(inputs["wo2"], f32).astype(BF16),
        "w_1": w1f,
        "w_2": np.asarray(inputs["w2"], f32).astype(BF16),
        "masks": _build_masks(T, pos0, 0, S),
        "ident_b": np.eye(128, dtype=BF16),
        "bq_c": col(np.asarray(inputs["bq"], f32) / np.sqrt(HD), KC),
        "bk_c": col(inputs["bk"], KC),
        "bi_c": col(inputs["bi"], KF),
        "b1_c": col(b1f, KF),
        "bv_r": np.asarray(inputs["bv"], f32).astype(BF16)[None, :],
        "bo_r": np.asarray(inputs["bo"], f32).astype(BF16)[None, :],
        "bo2_r": np.asarray(inputs["bo2"], f32).astype(BF16)[None, :],
        "b2_r": np.asarray(inputs["b2"], f32).astype(BF16)[None, :],
    }
    return im, b1f


def compute_flags(inputs, b1f):
    f32 = np.float32

    def nz(x):
        return bool(np.any(np.asarray(x, f32) != 0.0))

    def nt(s, b):  # affine non-trivial
        return bool(
            np.any(np.asarray(s, f32) != 1.0) or np.any(np.asarray(b, f32) != 0.0)
        )

    return {
        "bq": nz(inputs["bq"]),
        "bk": nz(inputs["bk"]),
        "bv": nz(inputs["bv"]),
        "bo": nz(inputs["bo"]),
        "bi": nz(inputs["bi"]),
        "bo2": nz(inputs["bo2"]),
        "b1": nz(b1f),
        "b2": nz(inputs["b2"]),
        "aff_n1": nt(inputs["n1_s"], inputs["n1_b"]),
        "aff_emb": nt(inputs["emb_ln_s"], inputs["emb_ln_b"]),
        "aff_attn": nt(inputs["attn_ln_s"], inputs["attn_ln_b"]),
        "aff_out": nt(inputs["out_ln_s"], inputs["out_ln_b"]),
    }


def _add_affine_inputs(im, inputs, flags):
    f32 = np.float32
    pairs = {
        "n1": ("n1_s", "n1_b"),
        "emb": ("emb_ln_s", "emb_ln_b"),
        "attn": ("attn_ln_s", "attn_ln_b"),
        "out": ("out_ln_s", "out_ln_b"),
    }
    for nm, (sk, bk_) in pairs.items():
        if flags[f"aff_{nm}"]:
            im[f"{nm}_s"] = np.tile(np.asarray(inputs[sk], f32)[None, :], (128, 1))
            im[f"{nm}_b"] = np.tile(np.asarray(inputs[bk_], f32)[None, :], (128, 1))
    return im


TRACE_RUN = False
LAST_RESULT = None


def kernel(**inputs):
    global LAST_RESULT
    _install_wait_split()
    from concourse.bass_utils import run_bass_kernel_spmd

    T = T_OWN
    in_maps = []
    flags = None
    for c in range(NCORES):
        im, b1f = _host_prep(inputs, T, c)
        if flags is None:
            flags = compute_flags(inputs, b1f)
        _add_affine_inputs(im, inputs, flags)
        in_maps.append(im)

    nc = build_nc(T, flags)
    res = run_bass_kernel_spmd(
        nc, in_maps, core_ids=list(range(NCORES)), trace=TRACE_RUN
    )
    LAST_RESULT = res

    out = np.zeros((B, S, D), np.float32)
    for c in range(NCORES):
        b = c // (NCORES // B)
        pos0 = (c % (NCORES // B)) * T
        out[b, pos0 : pos0 + T] = res.results[c]["out"]
    return out
